# revision 31
# baseline (speedup 1.0000x reference)
"""Trainium2 Bass kernel for nn_AsynchronousGRUActor.

Math (per agent row b):
    a  = tanh(x @ fc1_w.T + fc1_b)
    gi = a @ w_ih.T + b_ih ; gh = h @ w_hh.T + b_hh   (gates [r, z, n])
    r = sig(i_r + h_r); z = sig(i_z + h_z); n = tanh(i_n + r * h_n)
    h' = (1-z)*n + z*h ; h' = h where mask==0
    logits = h' @ fc2_w.T + fc2_b

Device design (pure data parallel over 8 cores, 62500 rows each):
  * Transposed layout: features on SBUF partitions, batch rows on the
    free dim. Host passes x.T / h.T, transposes outputs back.
  * bf16 storage + matmul operands (1 cycle/row on the PE vs 4 for
    fp32, half the DMA bytes); fp32 PSUM accumulation, ACT/DVE compute
    fp32 internally.
  * zc == 1-z form: h' = h - zc*(h - n), zc = sigmoid(-z_pre). Inactive
    agents get -BIG added to the zc pre-activation (K=1 matmul with a
    -1 stationary row against the maskbig row) so zc==0 and h' == h
    exactly.
  * Tile pairing: a 2048-row chunk is two PAIRS of 512-row tiles; the
    even tile of a pair lives on partitions 0:64, the odd on 64:128.
    ht2 holds h(even blocks) on top / h(odd blocks) on bottom via two
    strided DMAs (each h byte loaded once). All 64-row tensors become
    128-partition tensors, and the update chain (d/u/h') runs as
    [*, 1024] ops spanning the whole chunk group.
  * q-elimination: DVE writes t1 = r*hn into the in-PSUM bank, then the
    in matmul runs start=False and ACCUMULATES w_in@a on top.
  * Per-parity rz gate order ([zc|r] even, [r|zc] odd) keeps every
    SBUF*SBUF TensorTensor at equal base partitions (walrus rule);
    SBUF*PSUM ops are base-free.
  * in/hn matmuls use block-diagonal [128,128] lhsT (one matmul per
    pair); fc2 uses block-diagonal [128,32] lhsT (one matmul per pair,
    logits of the 4 tiles pack a [64,512] bank, decoded on host).
  * GPSIMD takes d2 and one of the two h' subtractions.
  * Emission is software-pipelined per chunk: FRONT(c), TAIL_B(c-2)
    [fc2/logits/newh DMA], TAIL_A(c) [in-mm/n/update chain] so engine
    queues always hold independent work ahead of late-chain ops.
"""

import numpy as np
import ml_dtypes

NCORES = 8
B, OBS, H, A3 = 500000, 128, 64, 16
BC = B // NCORES            # rows per core
NTILE = 512                 # rows per matmul stream (one PSUM bank)
CHUNK = 2048                # rows per chunk group (2 pairs of 2 tiles)
NPAIR = 2
NCHUNK_FULL = 31
RPAD = NCHUNK_FULL * CHUNK  # 63488 padded rows per core
BIG = 16384.0
BF16 = ml_dtypes.bfloat16

_CACHE = {}


def _build(rpad, use_bhn, bufs=None):
    import concourse.bacc as bacc
    import concourse.mybir as mybir
    import concourse.tile as tile

    DT = mybir.dt.bfloat16
    DT32 = mybir.dt.float32
    AF = mybir.ActivationFunctionType
    ALU = mybir.AluOpType
    nchunk = rpad // CHUNK
    GW = NPAIR * NTILE  # group width for [*, 1024] elementwise ops
    bufs = bufs or {}
    bf = lambda k, d: bufs.get(k, d)

    nc = bacc.Bacc(None, target_bir_lowering=False, debug=False)

    xT = nc.declare_dram_parameter("xT", [OBS, rpad], DT, isOutput=False)
    hT = nc.declare_dram_parameter("hT", [H + 1, rpad], DT, isOutput=False)
    w_fc1 = nc.declare_dram_parameter("w_fc1", [OBS, H], DT, isOutput=False)
    w_ihrz = nc.declare_dram_parameter("w_ihrz", [2 * H, 2 * H], DT, isOutput=False)
    w_hhrz = nc.declare_dram_parameter("w_hhrz", [2 * H, 2 * H], DT, isOutput=False)
    w_in = nc.declare_dram_parameter("w_in", [2 * H, 2 * H], DT, isOutput=False)
    w_hn = nc.declare_dram_parameter("w_hn", [2 * H, 2 * H], DT, isOutput=False)
    w_fc2 = nc.declare_dram_parameter("w_fc2", [2 * H, 32], DT, isOutput=False)
    b_a2 = nc.declare_dram_parameter("b_a2", [2 * H, 1], DT32, isOutput=False)
    b_rz_e = nc.declare_dram_parameter("b_rz_e", [2 * H, 1], DT32, isOutput=False)
    b_rz_o = nc.declare_dram_parameter("b_rz_o", [2 * H, 1], DT32, isOutput=False)
    b_in2 = nc.declare_dram_parameter("b_in2", [2 * H, 1], DT32, isOutput=False)
    if use_bhn:
        b_hn = nc.declare_dram_parameter("b_hn", [1, 2 * H], DT, isOutput=False)
    newhT = nc.declare_dram_parameter("newhT", [H, rpad], DT, isOutput=True)
    # packed logits: rows 16k:16k+16 hold tile (4c+k)'s logits for chunk c
    logitsP = nc.declare_dram_parameter("logitsP", [64, rpad // 4], DT, isOutput=True)

    with tile.TileContext(nc) as tc:
        with (
            tc.tile_pool(name="wp", bufs=1) as wp,
            tc.tile_pool(name="sbx", bufs=bf("sbx", 3)) as sbx,
            tc.tile_pool(name="sbh", bufs=bf("sbh", 3)) as sbh,
            tc.tile_pool(name="sbo", bufs=bf("sbo", 5)) as sbo,
            tc.tile_pool(name="sbm", bufs=bf("sbm", 3)) as sbm,
            tc.tile_pool(name="sb", bufs=bf("sb", 6)) as sb,
            tc.tile_pool(name="sblg", bufs=bf("sblg", 4)) as sblg,
            tc.tile_pool(name="psa2", bufs=bf("a2", 1), space="PSUM") as psa2,
            tc.tile_pool(name="psrze", bufs=bf("rze", 1), space="PSUM") as psrze,
            tc.tile_pool(name="psrzo", bufs=bf("rzo", 1), space="PSUM") as psrzo,
            tc.tile_pool(name="psin", bufs=bf("in", 2), space="PSUM") as psin,
            tc.tile_pool(name="pshn", bufs=bf("hn", 1), space="PSUM") as pshn,
            tc.tile_pool(name="psl", bufs=bf("l", 1), space="PSUM") as psl,
        ):
            wfc1_t = wp.tile([OBS, H], DT)
            wihrz_t = wp.tile([2 * H, 2 * H], DT)
            whhrz_t = wp.tile([2 * H, 2 * H], DT)
            win_t = wp.tile([2 * H, 2 * H], DT)
            whn_t = wp.tile([2 * H, 2 * H], DT)
            wfc2_t = wp.tile([2 * H, 32], DT)
            ba2_t = wp.tile([2 * H, 1], DT32)
            brze_t = wp.tile([2 * H, 1], DT32)
            brzo_t = wp.tile([2 * H, 1], DT32)
            bin2_t = wp.tile([2 * H, 1], DT32)
            onesneg_t = wp.tile([1, H], DT)
            nc.sync.dma_start(wfc1_t[:], w_fc1[:])
            nc.sync.dma_start(wihrz_t[:], w_ihrz[:])
            nc.sync.dma_start(whhrz_t[:], w_hhrz[:])
            nc.sync.dma_start(win_t[:], w_in[:])
            nc.sync.dma_start(whn_t[:], w_hn[:])
            nc.sync.dma_start(wfc2_t[:], w_fc2[:])
            nc.sync.dma_start(ba2_t[:], b_a2[:])
            nc.sync.dma_start(brze_t[:], b_rz_e[:])
            nc.sync.dma_start(brzo_t[:], b_rz_o[:])
            nc.sync.dma_start(bin2_t[:], b_in2[:])
            nc.gpsimd.memset(onesneg_t[:], -1.0)
            if use_bhn:
                bhn_t = wp.tile([1, 2 * H], DT)
                ones_t = wp.tile([1, NTILE], DT)
                nc.sync.dma_start(bhn_t[:], b_hn[:])
                nc.gpsimd.memset(ones_t[:], 1.0)

            pend_a = []  # tail_a closures (same-chunk late chain)
            pend_b = []  # tail_b closures (fc2/logits/newh, 2 chunks later)

            for c in range(nchunk):
                c0 = c * CHUNK
                xt = sbx.tile([OBS, CHUNK], DT)
                ht2 = sbh.tile([2 * H, CHUNK // 2], DT)
                mrow = sbm.tile([1, CHUNK], DT)
                nc.sync.dma_start(xt[:], xT[:, c0 : c0 + CHUNK])
                # striped h load: even 512-blocks -> ht2 top (pair-packed),
                # odd blocks -> bottom. ht2 col 512p:512(p+1) == pair p.
                src = hT[0:H, c0 : c0 + CHUNK].rearrange(
                    "p (np two n) -> p np two n", two=2, n=NTILE
                )
                dst_t = ht2[0:H, :].rearrange("p (np n) -> p np n", n=NTILE)
                dst_b = ht2[H : 2 * H, :].rearrange("p (np n) -> p np n", n=NTILE)
                nc.sync.dma_start(dst_t[:], src[:, :, 0, :])
                nc.sync.dma_start(dst_b[:], src[:, :, 1, :])
                nc.sync.dma_start(mrow[:], hT[H : H + 1, c0 : c0 + CHUNK])

                # ---- FRONT ----
                a2 = psa2.tile([2 * H, GW], DT32)
                for p in range(NPAIR):
                    pc = slice(p * NTILE, (p + 1) * NTILE)
                    nc.tensor.matmul(
                        a2[0:H, pc], wfc1_t[:], xt[:, 2 * p * NTILE : (2 * p + 1) * NTILE],
                        start=True, stop=True,
                    )
                    nc.tensor.matmul(
                        a2[H : 2 * H, pc], wfc1_t[:],
                        xt[:, (2 * p + 1) * NTILE : (2 * p + 2) * NTILE],
                        start=True, stop=True,
                    )
                a2s = sb.tile([2 * H, GW], DT)
                nc.scalar.activation(a2s[:], a2[:], AF.Tanh, bias=ba2_t[:])

                rzse = sb.tile([2 * H, GW], DT32)
                rzso = sb.tile([2 * H, GW], DT32)
                inps = []
                for p in range(NPAIR):
                    pc = slice(p * NTILE, (p + 1) * NTILE)
                    me = mrow[0:1, 2 * p * NTILE : (2 * p + 1) * NTILE]
                    mo = mrow[0:1, (2 * p + 1) * NTILE : (2 * p + 2) * NTILE]

                    rze = psrze.tile([2 * H, NTILE], DT32)
                    nc.tensor.matmul(rze[:], wihrz_t[0:H, :], a2s[0:H, pc], start=True, stop=False)
                    nc.tensor.matmul(rze[:], whhrz_t[0:H, :], ht2[0:H, pc], start=False, stop=True)
                    nc.tensor.matmul(
                        rze[0:H, :], onesneg_t[0:1, :], me,
                        start=False, stop=True, skip_group_check=True,
                    )
                    nc.scalar.activation(rzse[:, pc], rze[:], AF.Sigmoid, bias=brze_t[:])

                    rzo = psrzo.tile([2 * H, NTILE], DT32)
                    nc.tensor.matmul(
                        rzo[:], wihrz_t[H : 2 * H, :], a2s[H : 2 * H, pc], start=True, stop=False
                    )
                    nc.tensor.matmul(
                        rzo[:], whhrz_t[H : 2 * H, :], ht2[H : 2 * H, pc], start=False, stop=True
                    )
                    nc.tensor.matmul(
                        rzo[H : 2 * H, :], onesneg_t[0:1, :], mo,
                        start=False, stop=True, skip_group_check=True,
                    )
                    nc.scalar.activation(rzso[:, pc], rzo[:], AF.Sigmoid, bias=brzo_t[:])

                    hnp = pshn.tile([2 * H, NTILE], DT32)
                    nc.tensor.matmul(hnp[:], whn_t[:], ht2[:, pc], start=True, stop=True)
                    if use_bhn:
                        nc.tensor.matmul(
                            hnp[:], bhn_t[0:1, :], ones_t[0:1, :],
                            start=False, stop=True, skip_group_check=True,
                        )

                    inp = psin.tile([2 * H, NTILE], DT32)
                    # t1 = r * hn straight into the in-bank
                    nc.vector.tensor_tensor(inp[0:H, :], rzse[H : 2 * H, pc], hnp[0:H, :], ALU.mult)
                    nc.vector.tensor_tensor(
                        inp[H : 2 * H, :], rzso[0:H, pc], hnp[H : 2 * H, :], ALU.mult
                    )
                    inps.append(inp)

                nh2 = sbo.tile([2 * H, CHUNK // 2], DT)

                def tail_a(a2s=a2s, rzse=rzse, rzso=rzso, ht2=ht2, nh2=nh2, inps=inps):
                    n2s = sb.tile([2 * H, GW], DT32)
                    d2 = sb.tile([2 * H, GW], DT32)
                    u2 = sb.tile([2 * H, GW], DT32)
                    for p in range(NPAIR):
                        pc = slice(p * NTILE, (p + 1) * NTILE)
                        # in matmul accumulates w_in@a onto t1: q = t1 + in
                        nc.tensor.matmul(
                            inps[p][:], win_t[:], a2s[:, pc], start=False, stop=True,
                            skip_group_check=True,
                        )
                        nc.scalar.activation(n2s[:, pc], inps[p][:], AF.Tanh, bias=bin2_t[:])
                        nc.gpsimd.tensor_tensor(d2[:, pc], ht2[:, pc], n2s[:, pc], ALU.subtract)
                        nc.vector.tensor_tensor(
                            u2[0:H, pc], rzse[0:H, pc], d2[0:H, pc], ALU.mult
                        )
                        nc.vector.tensor_tensor(
                            u2[H : 2 * H, pc], rzso[H : 2 * H, pc], d2[H : 2 * H, pc], ALU.mult
                        )
                        nc.vector.tensor_tensor(
                            nh2[0:H, pc], ht2[0:H, pc], u2[0:H, pc], ALU.subtract
                        )
                        nc.gpsimd.tensor_tensor(
                            nh2[H : 2 * H, pc], ht2[H : 2 * H, pc], u2[H : 2 * H, pc], ALU.subtract
                        )

                def tail_b(nh2=nh2, c0=c0, c=c):
                    pl = psl.tile([2 * H, NTILE], DT32)
                    for p in range(NPAIR):
                        pc = slice(p * NTILE, (p + 1) * NTILE)
                        nc.tensor.matmul(
                            pl[32 * p : 32 * p + 32, :], wfc2_t[:], nh2[:, pc],
                            start=True, stop=True,
                        )
                    lg = sblg.tile([2 * H, NTILE], DT)
                    nc.vector.tensor_copy(lg[0:64, :], pl[0:64, :])
                    nc.sync.dma_start(logitsP[:, c * NTILE : (c + 1) * NTILE], lg[0:64, :])
                    dstg = newhT[:, c0 : c0 + CHUNK].rearrange(
                        "p (np two n) -> p np two n", two=2, n=NTILE
                    )
                    srcg_t = nh2[0:H, :].rearrange("p (np n) -> p np n", n=NTILE)
                    srcg_b = nh2[H : 2 * H, :].rearrange("p (np n) -> p np n", n=NTILE)
                    nc.sync.dma_start(dstg[:, :, 0, :], srcg_t[:])
                    nc.sync.dma_start(dstg[:, :, 1, :], srcg_b[:])

                if len(pend_b) > 1:
                    pend_b.pop(0)()
                if pend_a:
                    pend_a.pop(0)()
                pend_a.append(tail_a)
                pend_b.append(tail_b)

            while pend_a:
                pend_a.pop(0)()
            while pend_b:
                pend_b.pop(0)()

    nc.compile()
    return nc


def get_nc(rpad=RPAD, use_bhn=False, bufs=None):
    key = (rpad, use_bhn)
    if key not in _CACHE:
        _CACHE[key] = _build(rpad, use_bhn, bufs)
    return _CACHE[key]


def make_weights(fc1_w, fc1_b, w_ih, w_hh, b_ih, b_hh, fc2_w):
    f32 = np.float32
    c16 = lambda a: np.ascontiguousarray(a, dtype=f32).astype(BF16)
    bd = lambda w: np.block(
        [[w, np.zeros((H, w.shape[1]), f32)], [np.zeros((H, w.shape[1]), f32), w]]
    )  # block-diag

    def rz_pair(w):
        r, z = w[0:H].T.astype(f32), w[H : 2 * H].T.astype(f32)
        even = np.concatenate([-z, r], axis=1)  # [zc | r]
        odd = np.concatenate([r, -z], axis=1)   # [r | zc]
        return np.concatenate([even, odd], axis=0)  # [128, 128]

    br = (b_ih + b_hh)[0:H].astype(f32)
    bz = (b_ih + b_hh)[H : 2 * H].astype(f32)
    b_in1 = b_ih[2 * H : 3 * H].astype(f32)
    # fc2 block-diag: [128, 32]: cols 0:16 <- rows 0:64 (even tile),
    # cols 16:32 <- rows 64:128 (odd tile)
    fc2bd = np.zeros((2 * H, 32), f32)
    fc2bd[0:H, 0:A3] = fc2_w.T.astype(f32)
    fc2bd[H : 2 * H, A3 : 2 * A3] = fc2_w.T.astype(f32)
    return {
        "w_fc1": c16(fc1_w.T),
        "w_ihrz": c16(rz_pair(w_ih)),
        "w_hhrz": c16(rz_pair(w_hh)),
        "w_in": c16(bd(w_ih[2 * H : 3 * H].T.astype(f32))),
        "w_hn": c16(bd(w_hh[2 * H : 3 * H].T.astype(f32))),
        "w_fc2": c16(fc2bd),
        "b_a2": np.ascontiguousarray(np.tile(fc1_b.astype(f32), 2)[:, None]),
        "b_rz_e": np.ascontiguousarray(np.concatenate([-bz, br])[:, None]),
        "b_rz_o": np.ascontiguousarray(np.concatenate([br, -bz])[:, None]),
        "b_in2": np.ascontiguousarray(np.tile(b_in1, 2)[:, None]),
        "b_hn_vals": c16(np.tile(b_hh[2 * H : 3 * H].astype(f32), 2)[None, :]),
    }


def make_in_maps(x, hidden_states, active_masks, wd, use_bhn):
    maskbig = (BIG * (1.0 - (active_masks != 0))).astype(np.float32)
    in_maps = []
    for c in range(NCORES):
        s = slice(c * BC, (c + 1) * BC)
        xTc = np.zeros((OBS, RPAD), BF16)
        xTc[:, :BC] = x[s].T.astype(BF16)
        hTc = np.zeros((H + 1, RPAD), BF16)
        hTc[:H, :BC] = hidden_states[s].T.astype(BF16)
        hTc[H, :BC] = maskbig[s].astype(BF16)
        m = {"xT": xTc, "hT": hTc}
        m.update({k: v for k, v in wd.items() if k != "b_hn_vals"})
        if use_bhn:
            m["b_hn"] = wd["b_hn_vals"]
        in_maps.append(m)
    return in_maps


def decode_logits_sim(logitsP, rpad):
    """pl rows: pair p contributes rows 32p:32p+32 = [even(16) | odd(16)];
    so row 16k:16(k+1) of chunk c's 512-col block = tile (4c + k)."""
    nchunk = rpad // CHUNK
    a = np.asarray(logitsP, np.float32).reshape(4, A3, nchunk, NTILE)
    return a.transpose(2, 0, 3, 1).reshape(rpad, A3)


def decode_logits(logitsP):
    return decode_logits_sim(logitsP, RPAD)


def kernel(x, hidden_states, active_masks, fc1_w, fc1_b, w_ih, w_hh, b_ih, b_hh, fc2_w, fc2_b):
    from concourse.bass_utils import run_bass_kernel_spmd

    x = np.asarray(x, np.float32)
    hidden_states = np.asarray(hidden_states, np.float32)
    active_masks = np.asarray(active_masks)
    wd = make_weights(
        np.asarray(fc1_w, np.float32), np.asarray(fc1_b, np.float32),
        np.asarray(w_ih, np.float32), np.asarray(w_hh, np.float32),
        np.asarray(b_ih, np.float32), np.asarray(b_hh, np.float32),
        np.asarray(fc2_w, np.float32),
    )
    use_bhn = bool(np.any(wd["b_hn_vals"] != 0))
    nc = get_nc(RPAD, use_bhn)
    in_maps = make_in_maps(x, hidden_states, active_masks, wd, use_bhn)

    res = run_bass_kernel_spmd(nc, in_maps, core_ids=list(range(NCORES)))

    logits = np.empty((B, A3), np.float32)
    newh = np.empty((B, H), np.float32)
    for c in range(NCORES):
        s = slice(c * BC, (c + 1) * BC)
        logits[s] = decode_logits(res.results[c]["logitsP"])[:BC]
        newh[s] = res.results[c]["newhT"][:, :BC].T.astype(np.float32)
    logits += np.asarray(fc2_b, np.float32)[None, :]
    return logits, newh


# revision 32
# speedup vs baseline: 1.0318x; 1.0318x over previous
"""Trainium2 Bass kernel for nn_AsynchronousGRUActor.

Math (per agent row b):
    a  = tanh(x @ fc1_w.T + fc1_b)
    gi = a @ w_ih.T + b_ih ; gh = h @ w_hh.T + b_hh   (gates [r, z, n])
    r = sig(i_r + h_r); z = sig(i_z + h_z); n = tanh(i_n + r * h_n)
    h' = (1-z)*n + z*h ; h' = h where mask==0
    logits = h' @ fc2_w.T + fc2_b

Device design (pure data parallel over 8 cores, 62500 rows each):
  * Transposed layout: features on SBUF partitions, batch rows on the
    free dim. Host passes x.T / h.T, transposes outputs back.
  * bf16 storage + matmul operands (1 cycle/row on the PE vs 4 for
    fp32, half the DMA bytes); fp32 PSUM accumulation, ACT/DVE compute
    fp32 internally.
  * zc == 1-z form: h' = h - zc*(h - n), zc = sigmoid(-z_pre). Inactive
    agents get -BIG added to the zc pre-activation (K=1 matmul with a
    -1 stationary row against the maskbig row) so zc==0 and h' == h
    exactly.
  * Tile pairing: a 2048-row chunk is two PAIRS of 512-row tiles; the
    even tile of a pair lives on partitions 0:64, the odd on 64:128.
    ht2 holds h(even blocks) on top / h(odd blocks) on bottom via two
    strided DMAs (each h byte loaded once). All 64-row tensors become
    128-partition tensors, and the update chain (d/u/h') runs as
    [*, 1024] ops spanning the whole chunk group.
  * q-elimination: DVE writes t1 = r*hn into the in-PSUM bank, then the
    in matmul runs start=False and ACCUMULATES w_in@a on top.
  * Per-parity rz gate order ([zc|r] even, [r|zc] odd) keeps every
    SBUF*SBUF TensorTensor at equal base partitions (walrus rule);
    SBUF*PSUM ops are base-free.
  * in/hn matmuls use block-diagonal [128,128] lhsT (one matmul per
    pair); fc2 uses block-diagonal [128,32] lhsT (one matmul per pair,
    logits of the 4 tiles pack a [64,512] bank, decoded on host).
  * GPSIMD takes d2 and one of the two h' subtractions.
  * Emission is software-pipelined per chunk: FRONT(c), TAIL_B(c-2)
    [fc2/logits/newh DMA], TAIL_A(c) [in-mm/n/update chain] so engine
    queues always hold independent work ahead of late-chain ops.
"""

import numpy as np
import ml_dtypes

NCORES = 8
B, OBS, H, A3 = 500000, 128, 64, 16
BC = B // NCORES            # rows per core
NTILE = 512                 # rows per matmul stream (one PSUM bank)
CHUNK = 2048                # rows per chunk group (2 pairs of 2 tiles)
NPAIR = 2
NCHUNK_FULL = 31
RPAD = NCHUNK_FULL * CHUNK  # 63488 padded rows per core
BIG = 16384.0
BF16 = ml_dtypes.bfloat16

_CACHE = {}


def _build(rpad, use_bhn, bufs=None):
    import concourse.bacc as bacc
    import concourse.mybir as mybir
    import concourse.tile as tile

    DT = mybir.dt.bfloat16
    DT32 = mybir.dt.float32
    AF = mybir.ActivationFunctionType
    ALU = mybir.AluOpType
    nchunk = rpad // CHUNK
    GW = NPAIR * NTILE  # group width for [*, 1024] elementwise ops
    bufs = bufs or {}
    bf = lambda k, d: bufs.get(k, d)

    nc = bacc.Bacc(None, target_bir_lowering=False, debug=False)

    xT = nc.declare_dram_parameter("xT", [OBS, rpad], DT, isOutput=False)
    hT = nc.declare_dram_parameter("hT", [H + 1, rpad], DT, isOutput=False)
    w_fc1 = nc.declare_dram_parameter("w_fc1", [OBS, H], DT, isOutput=False)
    w_ihrz = nc.declare_dram_parameter("w_ihrz", [2 * H, 2 * H], DT, isOutput=False)
    w_hhrz = nc.declare_dram_parameter("w_hhrz", [2 * H, 2 * H], DT, isOutput=False)
    w_in = nc.declare_dram_parameter("w_in", [2 * H, 2 * H], DT, isOutput=False)
    w_hn = nc.declare_dram_parameter("w_hn", [2 * H, 2 * H], DT, isOutput=False)
    w_fc2 = nc.declare_dram_parameter("w_fc2", [2 * H, 32], DT, isOutput=False)
    b_a2 = nc.declare_dram_parameter("b_a2", [2 * H, 1], DT32, isOutput=False)
    b_rz_e = nc.declare_dram_parameter("b_rz_e", [2 * H, 1], DT32, isOutput=False)
    b_rz_o = nc.declare_dram_parameter("b_rz_o", [2 * H, 1], DT32, isOutput=False)
    b_in2 = nc.declare_dram_parameter("b_in2", [2 * H, 1], DT32, isOutput=False)
    if use_bhn:
        b_hn = nc.declare_dram_parameter("b_hn", [1, 2 * H], DT, isOutput=False)
    newhT = nc.declare_dram_parameter("newhT", [H, rpad], DT, isOutput=True)
    # packed logits: rows 16k:16k+16 hold tile (4c+k)'s logits for chunk c
    logitsP = nc.declare_dram_parameter("logitsP", [64, rpad // 4], DT, isOutput=True)

    with tile.TileContext(nc) as tc:
        with (
            tc.tile_pool(name="wp", bufs=1) as wp,
            tc.tile_pool(name="sbx", bufs=bf("sbx", 3)) as sbx,
            tc.tile_pool(name="sbh", bufs=bf("sbh", 3)) as sbh,
            tc.tile_pool(name="sbo", bufs=bf("sbo", 5)) as sbo,
            tc.tile_pool(name="sbm", bufs=bf("sbm", 3)) as sbm,
            tc.tile_pool(name="sb", bufs=bf("sb", 6)) as sb,
            tc.tile_pool(name="sblg", bufs=bf("sblg", 4)) as sblg,
            tc.tile_pool(name="psa2", bufs=bf("a2", 1), space="PSUM") as psa2,
            tc.tile_pool(name="psrze", bufs=bf("rze", 1), space="PSUM") as psrze,
            tc.tile_pool(name="psrzo", bufs=bf("rzo", 1), space="PSUM") as psrzo,
            tc.tile_pool(name="psin", bufs=bf("in", 2), space="PSUM") as psin,
            tc.tile_pool(name="pshn", bufs=bf("hn", 1), space="PSUM") as pshn,
            tc.tile_pool(name="psl", bufs=bf("l", 1), space="PSUM") as psl,
        ):
            wfc1_t = wp.tile([OBS, H], DT)
            wihrz_t = wp.tile([2 * H, 2 * H], DT)
            whhrz_t = wp.tile([2 * H, 2 * H], DT)
            win_t = wp.tile([2 * H, 2 * H], DT)
            whn_t = wp.tile([2 * H, 2 * H], DT)
            wfc2_t = wp.tile([2 * H, 32], DT)
            ba2_t = wp.tile([2 * H, 1], DT32)
            brze_t = wp.tile([2 * H, 1], DT32)
            brzo_t = wp.tile([2 * H, 1], DT32)
            bin2_t = wp.tile([2 * H, 1], DT32)
            onesneg_t = wp.tile([1, H], DT)
            nc.sync.dma_start(wfc1_t[:], w_fc1[:])
            nc.sync.dma_start(wihrz_t[:], w_ihrz[:])
            nc.sync.dma_start(whhrz_t[:], w_hhrz[:])
            nc.sync.dma_start(win_t[:], w_in[:])
            nc.sync.dma_start(whn_t[:], w_hn[:])
            nc.sync.dma_start(wfc2_t[:], w_fc2[:])
            nc.sync.dma_start(ba2_t[:], b_a2[:])
            nc.sync.dma_start(brze_t[:], b_rz_e[:])
            nc.sync.dma_start(brzo_t[:], b_rz_o[:])
            nc.sync.dma_start(bin2_t[:], b_in2[:])
            nc.gpsimd.memset(onesneg_t[:], -1.0)
            if use_bhn:
                bhn_t = wp.tile([1, 2 * H], DT)
                ones_t = wp.tile([1, NTILE], DT)
                nc.sync.dma_start(bhn_t[:], b_hn[:])
                nc.gpsimd.memset(ones_t[:], 1.0)

            pend_a = []  # tail_a closures (same-chunk late chain)
            pend_b = []  # tail_b closures (fc2/logits/newh, 2 chunks later)

            for c in range(nchunk):
                c0 = c * CHUNK
                xt = sbx.tile([OBS, CHUNK], DT)
                ht2 = sbh.tile([2 * H, CHUNK // 2], DT)
                mrow = sbm.tile([1, CHUNK], DT)
                nc.sync.dma_start(xt[:], xT[:, c0 : c0 + CHUNK])
                # striped h load: even 512-blocks -> ht2 top (pair-packed),
                # odd blocks -> bottom. ht2 col 512p:512(p+1) == pair p.
                src = hT[0:H, c0 : c0 + CHUNK].rearrange(
                    "p (np two n) -> p np two n", two=2, n=NTILE
                )
                dst_t = ht2[0:H, :].rearrange("p (np n) -> p np n", n=NTILE)
                dst_b = ht2[H : 2 * H, :].rearrange("p (np n) -> p np n", n=NTILE)
                nc.sync.dma_start(dst_t[:], src[:, :, 0, :])
                nc.sync.dma_start(dst_b[:], src[:, :, 1, :])
                nc.sync.dma_start(mrow[:], hT[H : H + 1, c0 : c0 + CHUNK])

                # ---- FRONT ----
                a2 = psa2.tile([2 * H, GW], DT32)
                for p in range(NPAIR):
                    pc = slice(p * NTILE, (p + 1) * NTILE)
                    nc.tensor.matmul(
                        a2[0:H, pc], wfc1_t[:], xt[:, 2 * p * NTILE : (2 * p + 1) * NTILE],
                        start=True, stop=True,
                    )
                    nc.tensor.matmul(
                        a2[H : 2 * H, pc], wfc1_t[:],
                        xt[:, (2 * p + 1) * NTILE : (2 * p + 2) * NTILE],
                        start=True, stop=True,
                    )
                a2s = sb.tile([2 * H, GW], DT)
                nc.scalar.activation(a2s[:], a2[:], AF.Tanh, bias=ba2_t[:])

                rzse = sb.tile([2 * H, GW], DT32)
                rzso = sb.tile([2 * H, GW], DT32)
                inps = []
                for p in range(NPAIR):
                    pc = slice(p * NTILE, (p + 1) * NTILE)
                    me = mrow[0:1, 2 * p * NTILE : (2 * p + 1) * NTILE]
                    mo = mrow[0:1, (2 * p + 1) * NTILE : (2 * p + 2) * NTILE]

                    rze = psrze.tile([2 * H, NTILE], DT32)
                    nc.tensor.matmul(rze[:], wihrz_t[0:H, :], a2s[0:H, pc], start=True, stop=False)
                    nc.tensor.matmul(rze[:], whhrz_t[0:H, :], ht2[0:H, pc], start=False, stop=True)
                    nc.tensor.matmul(
                        rze[0:H, :], onesneg_t[0:1, :], me,
                        start=False, stop=True, skip_group_check=True,
                    )
                    nc.scalar.activation(rzse[:, pc], rze[:], AF.Sigmoid, bias=brze_t[:])

                    rzo = psrzo.tile([2 * H, NTILE], DT32)
                    nc.tensor.matmul(
                        rzo[:], wihrz_t[H : 2 * H, :], a2s[H : 2 * H, pc], start=True, stop=False
                    )
                    nc.tensor.matmul(
                        rzo[:], whhrz_t[H : 2 * H, :], ht2[H : 2 * H, pc], start=False, stop=True
                    )
                    nc.tensor.matmul(
                        rzo[H : 2 * H, :], onesneg_t[0:1, :], mo,
                        start=False, stop=True, skip_group_check=True,
                    )
                    nc.scalar.activation(rzso[:, pc], rzo[:], AF.Sigmoid, bias=brzo_t[:])

                    hnp = pshn.tile([2 * H, NTILE], DT32)
                    nc.tensor.matmul(hnp[:], whn_t[:], ht2[:, pc], start=True, stop=True)
                    if use_bhn:
                        nc.tensor.matmul(
                            hnp[:], bhn_t[0:1, :], ones_t[0:1, :],
                            start=False, stop=True, skip_group_check=True,
                        )

                    inp = psin.tile([2 * H, NTILE], DT32)
                    # t1 = r * hn straight into the in-bank
                    nc.vector.tensor_tensor(inp[0:H, :], rzse[H : 2 * H, pc], hnp[0:H, :], ALU.mult)
                    nc.vector.tensor_tensor(
                        inp[H : 2 * H, :], rzso[0:H, pc], hnp[H : 2 * H, :], ALU.mult
                    )
                    inps.append(inp)

                nh2 = sbo.tile([2 * H, CHUNK // 2], DT)

                def tail_a(a2s=a2s, rzse=rzse, rzso=rzso, ht2=ht2, nh2=nh2, inps=inps):
                    n2s = sb.tile([2 * H, GW], DT32)
                    d2 = sb.tile([2 * H, GW], DT32)
                    u2 = sb.tile([2 * H, GW], DT32)
                    for p in range(NPAIR):
                        pc = slice(p * NTILE, (p + 1) * NTILE)
                        # in matmul accumulates w_in@a onto t1: q = t1 + in
                        nc.tensor.matmul(
                            inps[p][:], win_t[:], a2s[:, pc], start=False, stop=True,
                            skip_group_check=True,
                        )
                        nc.scalar.activation(n2s[:, pc], inps[p][:], AF.Tanh, bias=bin2_t[:])
                        nc.gpsimd.tensor_tensor(d2[:, pc], ht2[:, pc], n2s[:, pc], ALU.subtract)
                        nc.vector.tensor_tensor(
                            u2[0:H, pc], rzse[0:H, pc], d2[0:H, pc], ALU.mult
                        )
                        nc.vector.tensor_tensor(
                            u2[H : 2 * H, pc], rzso[H : 2 * H, pc], d2[H : 2 * H, pc], ALU.mult
                        )
                        nc.vector.tensor_tensor(
                            nh2[0:H, pc], ht2[0:H, pc], u2[0:H, pc], ALU.subtract
                        )
                        nc.gpsimd.tensor_tensor(
                            nh2[H : 2 * H, pc], ht2[H : 2 * H, pc], u2[H : 2 * H, pc], ALU.subtract
                        )

                def tail_b(nh2=nh2, c0=c0, c=c):
                    pl = psl.tile([2 * H, NTILE], DT32)
                    for p in range(NPAIR):
                        pc = slice(p * NTILE, (p + 1) * NTILE)
                        nc.tensor.matmul(
                            pl[32 * p : 32 * p + 32, :], wfc2_t[:], nh2[:, pc],
                            start=True, stop=True,
                        )
                    lg = sblg.tile([2 * H, NTILE], DT)
                    nc.vector.tensor_copy(lg[0:64, :], pl[0:64, :])
                    nc.sync.dma_start(logitsP[:, c * NTILE : (c + 1) * NTILE], lg[0:64, :])
                    dstg = newhT[:, c0 : c0 + CHUNK].rearrange(
                        "p (np two n) -> p np two n", two=2, n=NTILE
                    )
                    srcg_t = nh2[0:H, :].rearrange("p (np n) -> p np n", n=NTILE)
                    srcg_b = nh2[H : 2 * H, :].rearrange("p (np n) -> p np n", n=NTILE)
                    nc.sync.dma_start(dstg[:, :, 0, :], srcg_t[:])
                    nc.sync.dma_start(dstg[:, :, 1, :], srcg_b[:])

                if len(pend_b) > 2:
                    pend_b.pop(0)()
                if pend_a:
                    pend_a.pop(0)()
                pend_a.append(tail_a)
                pend_b.append(tail_b)

            while pend_a:
                pend_a.pop(0)()
            while pend_b:
                pend_b.pop(0)()

    nc.compile()
    return nc


def get_nc(rpad=RPAD, use_bhn=False, bufs=None):
    key = (rpad, use_bhn)
    if key not in _CACHE:
        _CACHE[key] = _build(rpad, use_bhn, bufs)
    return _CACHE[key]


def make_weights(fc1_w, fc1_b, w_ih, w_hh, b_ih, b_hh, fc2_w):
    f32 = np.float32
    c16 = lambda a: np.ascontiguousarray(a, dtype=f32).astype(BF16)
    bd = lambda w: np.block(
        [[w, np.zeros((H, w.shape[1]), f32)], [np.zeros((H, w.shape[1]), f32), w]]
    )  # block-diag

    def rz_pair(w):
        r, z = w[0:H].T.astype(f32), w[H : 2 * H].T.astype(f32)
        even = np.concatenate([-z, r], axis=1)  # [zc | r]
        odd = np.concatenate([r, -z], axis=1)   # [r | zc]
        return np.concatenate([even, odd], axis=0)  # [128, 128]

    br = (b_ih + b_hh)[0:H].astype(f32)
    bz = (b_ih + b_hh)[H : 2 * H].astype(f32)
    b_in1 = b_ih[2 * H : 3 * H].astype(f32)
    # fc2 block-diag: [128, 32]: cols 0:16 <- rows 0:64 (even tile),
    # cols 16:32 <- rows 64:128 (odd tile)
    fc2bd = np.zeros((2 * H, 32), f32)
    fc2bd[0:H, 0:A3] = fc2_w.T.astype(f32)
    fc2bd[H : 2 * H, A3 : 2 * A3] = fc2_w.T.astype(f32)
    return {
        "w_fc1": c16(fc1_w.T),
        "w_ihrz": c16(rz_pair(w_ih)),
        "w_hhrz": c16(rz_pair(w_hh)),
        "w_in": c16(bd(w_ih[2 * H : 3 * H].T.astype(f32))),
        "w_hn": c16(bd(w_hh[2 * H : 3 * H].T.astype(f32))),
        "w_fc2": c16(fc2bd),
        "b_a2": np.ascontiguousarray(np.tile(fc1_b.astype(f32), 2)[:, None]),
        "b_rz_e": np.ascontiguousarray(np.concatenate([-bz, br])[:, None]),
        "b_rz_o": np.ascontiguousarray(np.concatenate([br, -bz])[:, None]),
        "b_in2": np.ascontiguousarray(np.tile(b_in1, 2)[:, None]),
        "b_hn_vals": c16(np.tile(b_hh[2 * H : 3 * H].astype(f32), 2)[None, :]),
    }


def make_in_maps(x, hidden_states, active_masks, wd, use_bhn):
    maskbig = (BIG * (1.0 - (active_masks != 0))).astype(np.float32)
    in_maps = []
    for c in range(NCORES):
        s = slice(c * BC, (c + 1) * BC)
        xTc = np.zeros((OBS, RPAD), BF16)
        xTc[:, :BC] = x[s].T.astype(BF16)
        hTc = np.zeros((H + 1, RPAD), BF16)
        hTc[:H, :BC] = hidden_states[s].T.astype(BF16)
        hTc[H, :BC] = maskbig[s].astype(BF16)
        m = {"xT": xTc, "hT": hTc}
        m.update({k: v for k, v in wd.items() if k != "b_hn_vals"})
        if use_bhn:
            m["b_hn"] = wd["b_hn_vals"]
        in_maps.append(m)
    return in_maps


def decode_logits_sim(logitsP, rpad):
    """pl rows: pair p contributes rows 32p:32p+32 = [even(16) | odd(16)];
    so row 16k:16(k+1) of chunk c's 512-col block = tile (4c + k)."""
    nchunk = rpad // CHUNK
    a = np.asarray(logitsP, np.float32).reshape(4, A3, nchunk, NTILE)
    return a.transpose(2, 0, 3, 1).reshape(rpad, A3)


def decode_logits(logitsP):
    return decode_logits_sim(logitsP, RPAD)


def kernel(x, hidden_states, active_masks, fc1_w, fc1_b, w_ih, w_hh, b_ih, b_hh, fc2_w, fc2_b):
    from concourse.bass_utils import run_bass_kernel_spmd

    x = np.asarray(x, np.float32)
    hidden_states = np.asarray(hidden_states, np.float32)
    active_masks = np.asarray(active_masks)
    wd = make_weights(
        np.asarray(fc1_w, np.float32), np.asarray(fc1_b, np.float32),
        np.asarray(w_ih, np.float32), np.asarray(w_hh, np.float32),
        np.asarray(b_ih, np.float32), np.asarray(b_hh, np.float32),
        np.asarray(fc2_w, np.float32),
    )
    use_bhn = bool(np.any(wd["b_hn_vals"] != 0))
    nc = get_nc(RPAD, use_bhn)
    in_maps = make_in_maps(x, hidden_states, active_masks, wd, use_bhn)

    res = run_bass_kernel_spmd(nc, in_maps, core_ids=list(range(NCORES)))

    logits = np.empty((B, A3), np.float32)
    newh = np.empty((B, H), np.float32)
    for c in range(NCORES):
        s = slice(c * BC, (c + 1) * BC)
        logits[s] = decode_logits(res.results[c]["logitsP"])[:BC]
        newh[s] = res.results[c]["newhT"][:, :BC].T.astype(np.float32)
    logits += np.asarray(fc2_b, np.float32)[None, :]
    return logits, newh


# revision 33
# speedup vs baseline: 1.0361x; 1.0041x over previous
"""Trainium2 Bass kernel for nn_AsynchronousGRUActor.

Math (per agent row b):
    a  = tanh(x @ fc1_w.T + fc1_b)
    gi = a @ w_ih.T + b_ih ; gh = h @ w_hh.T + b_hh   (gates [r, z, n])
    r = sig(i_r + h_r); z = sig(i_z + h_z); n = tanh(i_n + r * h_n)
    h' = (1-z)*n + z*h ; h' = h where mask==0
    logits = h' @ fc2_w.T + fc2_b

Device design (pure data parallel over 8 cores, 62500 rows each):
  * Transposed layout: features on SBUF partitions, batch rows on the
    free dim. Host passes x.T / h.T, transposes outputs back.
  * bf16 storage + matmul operands (1 cycle/row on the PE vs 4 for
    fp32, half the DMA bytes); fp32 PSUM accumulation, ACT/DVE compute
    fp32 internally.
  * zc == 1-z form: h' = h - zc*(h - n), zc = sigmoid(-z_pre). Inactive
    agents get -BIG added to the zc pre-activation (K=1 matmul with a
    -1 stationary row against the maskbig row) so zc==0 and h' == h
    exactly.
  * Tile pairing: a 2048-row chunk is two PAIRS of 512-row tiles; the
    even tile of a pair lives on partitions 0:64, the odd on 64:128.
    ht2 holds h(even blocks) on top / h(odd blocks) on bottom via two
    strided DMAs (each h byte loaded once). All 64-row tensors become
    128-partition tensors, and the update chain (d/u/h') runs as
    [*, 1024] ops spanning the whole chunk group.
  * q-elimination: DVE writes t1 = r*hn into the in-PSUM bank, then the
    in matmul runs start=False and ACCUMULATES w_in@a on top.
  * Per-parity rz gate order ([zc|r] even, [r|zc] odd) keeps every
    SBUF*SBUF TensorTensor at equal base partitions (walrus rule);
    SBUF*PSUM ops are base-free.
  * in/hn matmuls use block-diagonal [128,128] lhsT (one matmul per
    pair); fc2 uses block-diagonal [128,32] lhsT (one matmul per pair,
    logits of the 4 tiles pack a [64,512] bank, decoded on host).
  * GPSIMD takes d2 and one of the two h' subtractions.
  * Emission is software-pipelined per chunk: FRONT(c), TAIL_B(c-2)
    [fc2/logits/newh DMA], TAIL_A(c) [in-mm/n/update chain] so engine
    queues always hold independent work ahead of late-chain ops.
"""

import numpy as np
import ml_dtypes

NCORES = 8
B, OBS, H, A3 = 500000, 128, 64, 16
BC = B // NCORES            # rows per core
NTILE = 512                 # rows per matmul stream (one PSUM bank)
CHUNK = 2048                # rows per chunk group (2 pairs of 2 tiles)
NPAIR = 2
NCHUNK_FULL = 31
RPAD = NCHUNK_FULL * CHUNK  # 63488 padded rows per core
BIG = 16384.0
BF16 = ml_dtypes.bfloat16

_CACHE = {}


def _build(rpad, use_bhn, bufs=None):
    import concourse.bacc as bacc
    import concourse.mybir as mybir
    import concourse.tile as tile

    DT = mybir.dt.bfloat16
    DT32 = mybir.dt.float32
    AF = mybir.ActivationFunctionType
    ALU = mybir.AluOpType
    nchunk = rpad // CHUNK
    GW = NPAIR * NTILE  # group width for [*, 1024] elementwise ops
    bufs = bufs or {}
    bf = lambda k, d: bufs.get(k, d)

    nc = bacc.Bacc(None, target_bir_lowering=False, debug=False)

    xT = nc.declare_dram_parameter("xT", [OBS, rpad], DT, isOutput=False)
    hT = nc.declare_dram_parameter("hT", [H + 1, rpad], DT, isOutput=False)
    w_fc1 = nc.declare_dram_parameter("w_fc1", [OBS, H], DT, isOutput=False)
    w_ihrz = nc.declare_dram_parameter("w_ihrz", [2 * H, 2 * H], DT, isOutput=False)
    w_hhrz = nc.declare_dram_parameter("w_hhrz", [2 * H, 2 * H], DT, isOutput=False)
    w_in = nc.declare_dram_parameter("w_in", [2 * H, 2 * H], DT, isOutput=False)
    w_hn = nc.declare_dram_parameter("w_hn", [2 * H, 2 * H], DT, isOutput=False)
    w_fc2 = nc.declare_dram_parameter("w_fc2", [2 * H, 32], DT, isOutput=False)
    b_a2 = nc.declare_dram_parameter("b_a2", [2 * H, 1], DT32, isOutput=False)
    b_rz_e = nc.declare_dram_parameter("b_rz_e", [2 * H, 1], DT32, isOutput=False)
    b_rz_o = nc.declare_dram_parameter("b_rz_o", [2 * H, 1], DT32, isOutput=False)
    b_in2 = nc.declare_dram_parameter("b_in2", [2 * H, 1], DT32, isOutput=False)
    if use_bhn:
        b_hn = nc.declare_dram_parameter("b_hn", [1, 2 * H], DT, isOutput=False)
    newhT = nc.declare_dram_parameter("newhT", [H, rpad], DT, isOutput=True)
    # packed logits: rows 16k:16k+16 hold tile (4c+k)'s logits for chunk c
    logitsP = nc.declare_dram_parameter("logitsP", [64, rpad // 4], DT, isOutput=True)

    with tile.TileContext(nc) as tc:
        with (
            tc.tile_pool(name="wp", bufs=1) as wp,
            tc.tile_pool(name="sbx", bufs=bf("sbx", 4)) as sbx,
            tc.tile_pool(name="sbh", bufs=bf("sbh", 4)) as sbh,
            tc.tile_pool(name="sbo", bufs=bf("sbo", 5)) as sbo,
            tc.tile_pool(name="sbm", bufs=bf("sbm", 4)) as sbm,
            tc.tile_pool(name="sb", bufs=bf("sb", 6)) as sb,
            tc.tile_pool(name="sblg", bufs=bf("sblg", 4)) as sblg,
            tc.tile_pool(name="psa2", bufs=bf("a2", 1), space="PSUM") as psa2,
            tc.tile_pool(name="psrze", bufs=bf("rze", 1), space="PSUM") as psrze,
            tc.tile_pool(name="psrzo", bufs=bf("rzo", 1), space="PSUM") as psrzo,
            tc.tile_pool(name="psin", bufs=bf("in", 2), space="PSUM") as psin,
            tc.tile_pool(name="pshn", bufs=bf("hn", 1), space="PSUM") as pshn,
            tc.tile_pool(name="psl", bufs=bf("l", 1), space="PSUM") as psl,
        ):
            wfc1_t = wp.tile([OBS, H], DT)
            wihrz_t = wp.tile([2 * H, 2 * H], DT)
            whhrz_t = wp.tile([2 * H, 2 * H], DT)
            win_t = wp.tile([2 * H, 2 * H], DT)
            whn_t = wp.tile([2 * H, 2 * H], DT)
            wfc2_t = wp.tile([2 * H, 32], DT)
            ba2_t = wp.tile([2 * H, 1], DT32)
            brze_t = wp.tile([2 * H, 1], DT32)
            brzo_t = wp.tile([2 * H, 1], DT32)
            bin2_t = wp.tile([2 * H, 1], DT32)
            onesneg_t = wp.tile([1, H], DT)
            nc.sync.dma_start(wfc1_t[:], w_fc1[:])
            nc.sync.dma_start(wihrz_t[:], w_ihrz[:])
            nc.sync.dma_start(whhrz_t[:], w_hhrz[:])
            nc.sync.dma_start(win_t[:], w_in[:])
            nc.sync.dma_start(whn_t[:], w_hn[:])
            nc.sync.dma_start(wfc2_t[:], w_fc2[:])
            nc.sync.dma_start(ba2_t[:], b_a2[:])
            nc.sync.dma_start(brze_t[:], b_rz_e[:])
            nc.sync.dma_start(brzo_t[:], b_rz_o[:])
            nc.sync.dma_start(bin2_t[:], b_in2[:])
            nc.gpsimd.memset(onesneg_t[:], -1.0)
            if use_bhn:
                bhn_t = wp.tile([1, 2 * H], DT)
                ones_t = wp.tile([1, NTILE], DT)
                nc.sync.dma_start(bhn_t[:], b_hn[:])
                nc.gpsimd.memset(ones_t[:], 1.0)

            pend_a = []  # tail_a closures (same-chunk late chain)
            pend_b = []  # tail_b closures (fc2/logits/newh, 2 chunks later)

            for c in range(nchunk):
                c0 = c * CHUNK
                xt = sbx.tile([OBS, CHUNK], DT)
                ht2 = sbh.tile([2 * H, CHUNK // 2], DT)
                mrow = sbm.tile([1, CHUNK], DT)
                nc.sync.dma_start(xt[:], xT[:, c0 : c0 + CHUNK])
                # striped h load: even 512-blocks -> ht2 top (pair-packed),
                # odd blocks -> bottom. ht2 col 512p:512(p+1) == pair p.
                src = hT[0:H, c0 : c0 + CHUNK].rearrange(
                    "p (np two n) -> p np two n", two=2, n=NTILE
                )
                dst_t = ht2[0:H, :].rearrange("p (np n) -> p np n", n=NTILE)
                dst_b = ht2[H : 2 * H, :].rearrange("p (np n) -> p np n", n=NTILE)
                nc.sync.dma_start(dst_t[:], src[:, :, 0, :])
                nc.sync.dma_start(dst_b[:], src[:, :, 1, :])
                nc.sync.dma_start(mrow[:], hT[H : H + 1, c0 : c0 + CHUNK])

                # ---- FRONT ----
                a2 = psa2.tile([2 * H, GW], DT32)
                for p in range(NPAIR):
                    pc = slice(p * NTILE, (p + 1) * NTILE)
                    nc.tensor.matmul(
                        a2[0:H, pc], wfc1_t[:], xt[:, 2 * p * NTILE : (2 * p + 1) * NTILE],
                        start=True, stop=True,
                    )
                    nc.tensor.matmul(
                        a2[H : 2 * H, pc], wfc1_t[:],
                        xt[:, (2 * p + 1) * NTILE : (2 * p + 2) * NTILE],
                        start=True, stop=True,
                    )
                a2s = sb.tile([2 * H, GW], DT)
                nc.scalar.activation(a2s[:], a2[:], AF.Tanh, bias=ba2_t[:])

                rzse = sb.tile([2 * H, GW], DT32)
                rzso = sb.tile([2 * H, GW], DT32)
                inps = []
                for p in range(NPAIR):
                    pc = slice(p * NTILE, (p + 1) * NTILE)
                    me = mrow[0:1, 2 * p * NTILE : (2 * p + 1) * NTILE]
                    mo = mrow[0:1, (2 * p + 1) * NTILE : (2 * p + 2) * NTILE]

                    rze = psrze.tile([2 * H, NTILE], DT32)
                    nc.tensor.matmul(rze[:], wihrz_t[0:H, :], a2s[0:H, pc], start=True, stop=False)
                    nc.tensor.matmul(rze[:], whhrz_t[0:H, :], ht2[0:H, pc], start=False, stop=True)
                    nc.tensor.matmul(
                        rze[0:H, :], onesneg_t[0:1, :], me,
                        start=False, stop=True, skip_group_check=True,
                    )
                    nc.scalar.activation(rzse[:, pc], rze[:], AF.Sigmoid, bias=brze_t[:])

                    rzo = psrzo.tile([2 * H, NTILE], DT32)
                    nc.tensor.matmul(
                        rzo[:], wihrz_t[H : 2 * H, :], a2s[H : 2 * H, pc], start=True, stop=False
                    )
                    nc.tensor.matmul(
                        rzo[:], whhrz_t[H : 2 * H, :], ht2[H : 2 * H, pc], start=False, stop=True
                    )
                    nc.tensor.matmul(
                        rzo[H : 2 * H, :], onesneg_t[0:1, :], mo,
                        start=False, stop=True, skip_group_check=True,
                    )
                    nc.scalar.activation(rzso[:, pc], rzo[:], AF.Sigmoid, bias=brzo_t[:])

                    hnp = pshn.tile([2 * H, NTILE], DT32)
                    nc.tensor.matmul(hnp[:], whn_t[:], ht2[:, pc], start=True, stop=True)
                    if use_bhn:
                        nc.tensor.matmul(
                            hnp[:], bhn_t[0:1, :], ones_t[0:1, :],
                            start=False, stop=True, skip_group_check=True,
                        )

                    inp = psin.tile([2 * H, NTILE], DT32)
                    # t1 = r * hn straight into the in-bank
                    nc.vector.tensor_tensor(inp[0:H, :], rzse[H : 2 * H, pc], hnp[0:H, :], ALU.mult)
                    nc.vector.tensor_tensor(
                        inp[H : 2 * H, :], rzso[0:H, pc], hnp[H : 2 * H, :], ALU.mult
                    )
                    inps.append(inp)

                nh2 = sbo.tile([2 * H, CHUNK // 2], DT)

                def tail_a(a2s=a2s, rzse=rzse, rzso=rzso, ht2=ht2, nh2=nh2, inps=inps):
                    n2s = sb.tile([2 * H, GW], DT32)
                    d2 = sb.tile([2 * H, GW], DT32)
                    u2 = sb.tile([2 * H, GW], DT32)
                    for p in range(NPAIR):
                        pc = slice(p * NTILE, (p + 1) * NTILE)
                        # in matmul accumulates w_in@a onto t1: q = t1 + in
                        nc.tensor.matmul(
                            inps[p][:], win_t[:], a2s[:, pc], start=False, stop=True,
                            skip_group_check=True,
                        )
                        nc.scalar.activation(n2s[:, pc], inps[p][:], AF.Tanh, bias=bin2_t[:])
                        nc.gpsimd.tensor_tensor(d2[:, pc], ht2[:, pc], n2s[:, pc], ALU.subtract)
                        nc.vector.tensor_tensor(
                            u2[0:H, pc], rzse[0:H, pc], d2[0:H, pc], ALU.mult
                        )
                        nc.vector.tensor_tensor(
                            u2[H : 2 * H, pc], rzso[H : 2 * H, pc], d2[H : 2 * H, pc], ALU.mult
                        )
                        nc.vector.tensor_tensor(
                            nh2[0:H, pc], ht2[0:H, pc], u2[0:H, pc], ALU.subtract
                        )
                        nc.gpsimd.tensor_tensor(
                            nh2[H : 2 * H, pc], ht2[H : 2 * H, pc], u2[H : 2 * H, pc], ALU.subtract
                        )

                def tail_b(nh2=nh2, c0=c0, c=c):
                    pl = psl.tile([2 * H, NTILE], DT32)
                    for p in range(NPAIR):
                        pc = slice(p * NTILE, (p + 1) * NTILE)
                        nc.tensor.matmul(
                            pl[32 * p : 32 * p + 32, :], wfc2_t[:], nh2[:, pc],
                            start=True, stop=True,
                        )
                    lg = sblg.tile([2 * H, NTILE], DT)
                    nc.vector.tensor_copy(lg[0:64, :], pl[0:64, :])
                    nc.sync.dma_start(logitsP[:, c * NTILE : (c + 1) * NTILE], lg[0:64, :])
                    dstg = newhT[:, c0 : c0 + CHUNK].rearrange(
                        "p (np two n) -> p np two n", two=2, n=NTILE
                    )
                    srcg_t = nh2[0:H, :].rearrange("p (np n) -> p np n", n=NTILE)
                    srcg_b = nh2[H : 2 * H, :].rearrange("p (np n) -> p np n", n=NTILE)
                    nc.sync.dma_start(dstg[:, :, 0, :], srcg_t[:])
                    nc.sync.dma_start(dstg[:, :, 1, :], srcg_b[:])

                if len(pend_b) > 2:
                    pend_b.pop(0)()
                if pend_a:
                    pend_a.pop(0)()
                pend_a.append(tail_a)
                pend_b.append(tail_b)

            while pend_a:
                pend_a.pop(0)()
            while pend_b:
                pend_b.pop(0)()

    nc.compile()
    return nc


def get_nc(rpad=RPAD, use_bhn=False, bufs=None):
    key = (rpad, use_bhn)
    if key not in _CACHE:
        _CACHE[key] = _build(rpad, use_bhn, bufs)
    return _CACHE[key]


def make_weights(fc1_w, fc1_b, w_ih, w_hh, b_ih, b_hh, fc2_w):
    f32 = np.float32
    c16 = lambda a: np.ascontiguousarray(a, dtype=f32).astype(BF16)
    bd = lambda w: np.block(
        [[w, np.zeros((H, w.shape[1]), f32)], [np.zeros((H, w.shape[1]), f32), w]]
    )  # block-diag

    def rz_pair(w):
        r, z = w[0:H].T.astype(f32), w[H : 2 * H].T.astype(f32)
        even = np.concatenate([-z, r], axis=1)  # [zc | r]
        odd = np.concatenate([r, -z], axis=1)   # [r | zc]
        return np.concatenate([even, odd], axis=0)  # [128, 128]

    br = (b_ih + b_hh)[0:H].astype(f32)
    bz = (b_ih + b_hh)[H : 2 * H].astype(f32)
    b_in1 = b_ih[2 * H : 3 * H].astype(f32)
    # fc2 block-diag: [128, 32]: cols 0:16 <- rows 0:64 (even tile),
    # cols 16:32 <- rows 64:128 (odd tile)
    fc2bd = np.zeros((2 * H, 32), f32)
    fc2bd[0:H, 0:A3] = fc2_w.T.astype(f32)
    fc2bd[H : 2 * H, A3 : 2 * A3] = fc2_w.T.astype(f32)
    return {
        "w_fc1": c16(fc1_w.T),
        "w_ihrz": c16(rz_pair(w_ih)),
        "w_hhrz": c16(rz_pair(w_hh)),
        "w_in": c16(bd(w_ih[2 * H : 3 * H].T.astype(f32))),
        "w_hn": c16(bd(w_hh[2 * H : 3 * H].T.astype(f32))),
        "w_fc2": c16(fc2bd),
        "b_a2": np.ascontiguousarray(np.tile(fc1_b.astype(f32), 2)[:, None]),
        "b_rz_e": np.ascontiguousarray(np.concatenate([-bz, br])[:, None]),
        "b_rz_o": np.ascontiguousarray(np.concatenate([br, -bz])[:, None]),
        "b_in2": np.ascontiguousarray(np.tile(b_in1, 2)[:, None]),
        "b_hn_vals": c16(np.tile(b_hh[2 * H : 3 * H].astype(f32), 2)[None, :]),
    }


def make_in_maps(x, hidden_states, active_masks, wd, use_bhn):
    maskbig = (BIG * (1.0 - (active_masks != 0))).astype(np.float32)
    in_maps = []
    for c in range(NCORES):
        s = slice(c * BC, (c + 1) * BC)
        xTc = np.zeros((OBS, RPAD), BF16)
        xTc[:, :BC] = x[s].T.astype(BF16)
        hTc = np.zeros((H + 1, RPAD), BF16)
        hTc[:H, :BC] = hidden_states[s].T.astype(BF16)
        hTc[H, :BC] = maskbig[s].astype(BF16)
        m = {"xT": xTc, "hT": hTc}
        m.update({k: v for k, v in wd.items() if k != "b_hn_vals"})
        if use_bhn:
            m["b_hn"] = wd["b_hn_vals"]
        in_maps.append(m)
    return in_maps


def decode_logits_sim(logitsP, rpad):
    """pl rows: pair p contributes rows 32p:32p+32 = [even(16) | odd(16)];
    so row 16k:16(k+1) of chunk c's 512-col block = tile (4c + k)."""
    nchunk = rpad // CHUNK
    a = np.asarray(logitsP, np.float32).reshape(4, A3, nchunk, NTILE)
    return a.transpose(2, 0, 3, 1).reshape(rpad, A3)


def decode_logits(logitsP):
    return decode_logits_sim(logitsP, RPAD)


def kernel(x, hidden_states, active_masks, fc1_w, fc1_b, w_ih, w_hh, b_ih, b_hh, fc2_w, fc2_b):
    from concourse.bass_utils import run_bass_kernel_spmd

    x = np.asarray(x, np.float32)
    hidden_states = np.asarray(hidden_states, np.float32)
    active_masks = np.asarray(active_masks)
    wd = make_weights(
        np.asarray(fc1_w, np.float32), np.asarray(fc1_b, np.float32),
        np.asarray(w_ih, np.float32), np.asarray(w_hh, np.float32),
        np.asarray(b_ih, np.float32), np.asarray(b_hh, np.float32),
        np.asarray(fc2_w, np.float32),
    )
    use_bhn = bool(np.any(wd["b_hn_vals"] != 0))
    nc = get_nc(RPAD, use_bhn)
    in_maps = make_in_maps(x, hidden_states, active_masks, wd, use_bhn)

    res = run_bass_kernel_spmd(nc, in_maps, core_ids=list(range(NCORES)))

    logits = np.empty((B, A3), np.float32)
    newh = np.empty((B, H), np.float32)
    for c in range(NCORES):
        s = slice(c * BC, (c + 1) * BC)
        logits[s] = decode_logits(res.results[c]["logitsP"])[:BC]
        newh[s] = res.results[c]["newhT"][:, :BC].T.astype(np.float32)
    logits += np.asarray(fc2_b, np.float32)[None, :]
    return logits, newh
